# revision 3
# baseline (speedup 1.0000x reference)
"""Trainium2 Bass kernel for a 3-layer stacked GRU + dual masked-linear heads.

Model (PyTorch GRUCell semantics, eval mode):
    h1,h2,h3 : 3 chained GRUCell layers over T=512 steps (B=32, F_IN=513, H=512)
    s1 = relu(h3_seq @ W_l1.T + b_l1); s2 = relu(h3_seq @ W_l2.T + b_l2)
    m1 = s1/(s1+s2+1e-16); m2 = s2/(s1+s2+1e-16)
    returns (m1*x, m2*x)

Device strategy:
  - "L-layout": per-step tensors as [128, 384] tiles; partition p=32q+b
    (q = H quarter, b = batch), free = 128g+j (g = gate r/z/n, j = H offset).
  - Matmuls keep h^T stationary, stream weights through 4 concurrently-
    running column groups; the q loop is INNERMOST and P1/P2 rounds are
    interleaved so consecutive PE instructions never serialize on one
    column group or PSUM bank.
  - 3-layer wavefront: superstep s runs layer l at t=s-l+1.  Each layer's
    hL->hT transpose is emitted at the START of the NEXT superstep's layer
    section (right before the P2 rounds that don't depend on it), so the
    strict-FIFO PE never stalls waiting for the DVE gate chain.
  - Biases enter via K=1 f16 matmul rounds: P1's round carries
    (b_ih+b_hh)_rz | b_hh_n merged; P2's round covers only the n columns.
  - h update uses h = n + z*(h_prev - n) (3 DVE ops).
  - The whole recurrence is replicated on every core; the output phase
    uses reciprocal_approx_fast + scalar_tensor_tensor.
"""

import os
import numpy as np

B, T, F, H = 32, 512, 513, 512
NCORES = 8

_CACHE = {}


# ---------------------------------------------------------------------------
# Host-side weight/input repacking (pure layout, no math beyond bias folding)
# ---------------------------------------------------------------------------

def _moving(W):
    """W [3H, K] (K multiple of 128) -> [128, KT, 4, 384] moving-operand tiles.

    out[kk, kt, q, 128*g+j] = W[g*512 + 128*q + j, 128*kt + kk]
    """
    K = W.shape[1]
    KT = K // 128
    Wk = W.reshape(3, 4, 128, KT, 128)  # [g, q, j, kt, kk]
    return np.ascontiguousarray(np.transpose(Wk, (4, 3, 1, 0, 2)).reshape(128, KT, 4, 384).astype(np.float16))


def _gate_rows(v):
    """v [3H] -> [4, 384] in (q, 128g+j) order."""
    return np.ascontiguousarray(np.transpose(np.asarray(v, np.float32).reshape(3, 4, 128), (1, 0, 2)).reshape(4, 384))


def prep_inputs(inputs, t_steps):
    x = np.asarray(inputs["x"], np.float32)
    t = t_steps
    p = {}

    # Recurrent (h -> gates) weights, 3 layers stacked: [128, 3, 4, 4, 384]
    p["Whm"] = np.ascontiguousarray(np.stack(
        [_moving(np.asarray(inputs[f"W_hh{l}"], np.float32)) for l in (1, 2, 3)], axis=1))
    # Input (h_prev -> gates) weights for layers 2,3: [128, 2, 4, 4, 384]
    p["Wim"] = np.ascontiguousarray(np.stack(
        [_moving(np.asarray(inputs[f"W_ih{l}"], np.float32)) for l in (2, 3)], axis=1))

    # Layer-1 x weights: [128, 5, 4, 384]; k-tile 4 packs [W[:,512]; b_ih1]
    W1 = np.asarray(inputs["W_ih1"], np.float32)
    Wxm = np.zeros((128, 5, 4, 384), np.float16)
    Wxm[:, :4] = _moving(W1[:, :512])
    Wxm[0, 4] = _gate_rows(W1[:, 512])
    Wxm[1, 4] = _gate_rows(inputs["b_ih1"])
    p["Wxm"] = Wxm

    # Bias rows (f16 K=1 moving operands), 32-partition-aligned.
    # blk 0 = P1-side row for layer l at partition 32*(l-1):
    #   L1: b_hh1 (b_ih1 rides the x k-tile); L2/L3: (b_ih+b_hh)_rz | b_hh_n
    # blk 1 = P2-side rows (b_ih, only n cols consumed): L2 at 0, L3 at 32.
    def row_p1(l):
        out = _gate_rows(inputs[f"b_hh{l}"]).copy()
        if l > 1:
            out[:, 0:256] += _gate_rows(inputs[f"b_ih{l}"])[:, 0:256]
        return out

    bias = np.zeros((128, 2, 1536), np.float16)
    for li, l in enumerate((1, 2, 3)):
        bias[32 * li, 0] = row_p1(l).reshape(-1)
    for li, l in enumerate((2, 3)):
        bias[32 * li, 1] = _gate_rows(inputs[f"b_ih{l}"]).reshape(-1)
    p["bias"] = bias

    # Pre-transposed x stream tiles [T, 128, 5, 32]:
    #   [t, kk, kt, b] = x[b, t, 128*kt+kk] (kt<4); kt=4: row0 = x[b,t,512], row1 = 1.0
    xT = np.zeros((t, 128, 5, 32), np.float16)
    xT[:, :, :4] = np.transpose(x[:, :t, :512].reshape(B, t, 4, 128), (1, 3, 2, 0))
    xT[:, 0, 4] = x[:, :t, 512].T
    xT[:, 1, 4] = 1.0
    p["xT"] = np.ascontiguousarray(xT)

    # Output head weights [128, 2, 4, 513]: [kk, head, kt, f] = W_l[f, 128kt+kk]
    p["WlT"] = np.ascontiguousarray(np.stack(
        [np.transpose(np.asarray(inputs[f"W_l{i}"], np.float32).reshape(513, 4, 128), (2, 1, 0))
         for i in (1, 2)], axis=1).astype(np.float32))
    # Head biases [128, 2, 5]: [pp, head, m] = b_l[m*128+pp]  (padded to 640)
    bl = np.zeros((128, 2, 5), np.float32)
    for i in (1, 2):
        bp = np.zeros(640, np.float32)
        bp[:513] = np.asarray(inputs[f"b_l{i}"], np.float32)
        bl[:, i - 1, :] = bp.reshape(5, 128).T
    p["bl"] = bl

    # x for the output masking, f-major [5, 128, T, B] (padded f to 640)
    xo = np.zeros((5, 128, t, B), np.float32)
    xo.reshape(640, t, B)[:513] = np.transpose(x[:, :t, :], (2, 1, 0))
    p["xo"] = xo
    return p


# ---------------------------------------------------------------------------
# Device kernel
# ---------------------------------------------------------------------------

def build_nc(t_steps, shard_output):
    from contextlib import ExitStack
    import concourse.bacc as bacc
    import concourse.bass as bass
    import concourse.mybir as mybir
    import concourse.tile as tile
    from concourse.masks import make_identity

    f32 = mybir.dt.float32
    f32r = mybir.dt.float32r
    f16 = mybir.dt.float16
    AF = mybir.ActivationFunctionType
    ALU = mybir.AluOpType

    t_total = t_steps
    nc = bacc.Bacc("TRN2", target_bir_lowering=False)

    # ---- DRAM I/O -------------------------------------------------------
    xT_d = nc.dram_tensor("xT", [t_total, 128, 5, 32], f16, kind="ExternalInput")
    Whm_d = nc.dram_tensor("Whm", [128, 3, 4, 4, 384], f16, kind="ExternalInput")
    Wim_d = nc.dram_tensor("Wim", [128, 2, 4, 4, 384], f16, kind="ExternalInput")
    Wxm_d = nc.dram_tensor("Wxm", [128, 5, 4, 384], f16, kind="ExternalInput")
    bias_d = nc.dram_tensor("bias", [128, 2, 1536], f16, kind="ExternalInput")
    WlT_d = nc.dram_tensor("WlT", [128, 2, 4, 513], f32r, kind="ExternalInput")
    bl_d = nc.dram_tensor("bl", [128, 2, 5], f32, kind="ExternalInput")
    xo_d = nc.dram_tensor("xo", [5, 128, t_total, B], f32, kind="ExternalInput")
    # outputs in f-major layout [5, 128, T, B] (host transposes to [B, T, F])
    out1_d = nc.dram_tensor("out1", [5, 128, t_total, B], f32, kind="ExternalOutput")
    out2_d = nc.dram_tensor("out2", [5, 128, t_total, B], f32, kind="ExternalOutput")

    with ExitStack() as ctx:
        tc = ctx.enter_context(tile.TileContext(nc))

        consts = ctx.enter_context(tc.tile_pool(name="consts", bufs=1))
        ident = consts.tile([128, 128], f32)
        make_identity(nc, ident)
        ones = consts.tile([128, 128], f16)
        nc.vector.memset(ones, 1.0)

        # DRAM scratch for the h3 transpose stream (consumed by output phase)
        dram = ctx.enter_context(tc.tile_pool(name="dram", bufs=1, space="DRAM"))
        h3T = dram.tile([t_total, 128, 128], f32r)

        with ExitStack() as rctx:
            wrec = rctx.enter_context(tc.tile_pool(name="wrec", bufs=1))
            Whm = wrec.tile([128, 3, 4, 4, 384], f16)
            nc.sync.dma_start(out=Whm, in_=Whm_d[:, :, :, :, :])
            Wim = wrec.tile([128, 2, 4, 4, 384], f16)
            nc.sync.dma_start(out=Wim, in_=Wim_d[:, :, :, :, :])
            Wxm = wrec.tile([128, 5, 4, 384], f16)
            nc.sync.dma_start(out=Wxm, in_=Wxm_d[:, :, :, :])
            bias_sb = wrec.tile([128, 2, 1536], f16)
            nc.sync.dma_start(out=bias_sb, in_=bias_d[:, :, :])

            xpool = rctx.enter_context(tc.tile_pool(name="xpool", bufs=4))
            hpool = rctx.enter_context(tc.tile_pool(name="hpool", bufs=2))
            gpool = rctx.enter_context(tc.tile_pool(name="gpool", bufs=2))
            p1p = rctx.enter_context(tc.tile_pool(name="p1p", bufs=1, space="PSUM"))
            p2p = rctx.enter_context(tc.tile_pool(name="p2p", bufs=1, space="PSUM"))
            tpp = rctx.enter_context(tc.tile_pool(name="tpp", bufs=2, space="PSUM"))

            def emit_transpose(l, t, hL):
                """hL [B-major] -> hT [H-major] via PE transpose + copy."""
                tp = tpp.tile([128, 128], f32, tag="tp")
                nc.tensor.transpose(tp, hL, ident)
                hT_new = hpool.tile([128, 128], f16, tag=f"hT_{l}")
                nc.scalar.copy(hT_new, tp)
                if l == 3:
                    h3row = hpool.tile([128, 128], f32r, tag="h3row")
                    nc.scalar.copy(h3row, tp)
                    nc.sync.dma_start(out=h3T[t, :, :], in_=h3row)
                return hT_new

            def emit_groups(l, t, src_hT, own_hT, own_hL):
                """Matmul groups + gate math for layer l, time t. Returns hL."""
                li = l - 1
                P1 = p1p.tile([128, 384], f32, tag=f"p1_{l}")
                P2 = p2p.tile([128, 384], f32, tag=f"p2_{l}")

                # Build per-round emitters; each round = 4 col-group matmuls.
                if l == 1:
                    xt = xpool.tile([128, 5, 32], f16)
                    nc.sync.dma_start(out=xt, in_=xT_d[t, :, :, :])

                    def mk_p2(kt):
                        def r():
                            kk = 128 if kt < 4 else 2
                            for q in range(4):
                                nc.tensor.matmul(
                                    P2[32 * q:32 * q + 32, :], xt[:kk, kt, :],
                                    Wxm[:kk, kt, q, :],
                                    start=(kt == 0), stop=(kt == 4),
                                    tile_position=(0, 32 * q))
                        return r
                    p2_rounds = [mk_p2(kt) for kt in range(5)]
                else:
                    def bias_p2():
                        # start=True clears the whole bank; weight rounds run
                        # start=False and overwrite-on-first-touch in rz cols.
                        rp = 32 * (li - 1)
                        for q in range(4):
                            nc.tensor.matmul(
                                P2[32 * q:32 * q + 32, 256:384],
                                ones[rp:rp + 1, 32 * q:32 * q + 32],
                                bias_sb[rp:rp + 1, 1, 384 * q + 256:384 * q + 384],
                                start=True, stop=False, tile_position=(rp, 32 * q))

                    def mk_p2(kt):
                        def r():
                            for q in range(4):
                                nc.tensor.matmul(
                                    P2[32 * q:32 * q + 32, :],
                                    src_hT[:, 32 * kt:32 * kt + 32],
                                    Wim[:, li - 1, kt, q, :],
                                    start=False, stop=(kt == 3),
                                    tile_position=(0, 32 * q))
                        return r
                    p2_rounds = [bias_p2] + [mk_p2(kt) for kt in range(4)]

                def bias_p1():
                    rp = 32 * li
                    for q in range(4):
                        nc.tensor.matmul(
                            P1[32 * q:32 * q + 32, :],
                            ones[rp:rp + 1, 32 * q:32 * q + 32],
                            bias_sb[rp:rp + 1, 0, 384 * q:384 * q + 384],
                            start=True, stop=(t == 0), tile_position=(rp, 32 * q))
                p1_rounds = [bias_p1]
                if t > 0:
                    def mk_p1(kt):
                        def r():
                            for q in range(4):
                                nc.tensor.matmul(
                                    P1[32 * q:32 * q + 32, :],
                                    own_hT[:, 32 * kt:32 * kt + 32],
                                    Whm[:, li, kt, q, :],
                                    start=False, stop=(kt == 3),
                                    tile_position=(0, 32 * q))
                        return r
                    p1_rounds += [mk_p1(kt) for kt in range(4)]

                # Interleave P1/P2 rounds: consecutive rounds target
                # different PSUM banks and stationaries.
                for i in range(max(len(p1_rounds), len(p2_rounds))):
                    if i < len(p2_rounds):
                        p2_rounds[i]()
                    if i < len(p1_rounds):
                        p1_rounds[i]()

                # ---- gates ----
                # a TensorTensor may read at most one PSUM operand; stage P2's
                # r|z half through SBUF (it is off the critical gh chain)
                g2 = gpool.tile([128, 256], f32, tag=f"dd_{l}")
                nc.scalar.copy(g2, P2[:, 0:256])
                rz = gpool.tile([128, 256], f32, tag=f"ca_{l}")
                nc.vector.tensor_add(rz, P1[:, 0:256], g2)
                nc.scalar.activation(rz, rz, AF.Sigmoid)
                z = rz[:, 128:256]

                rn = gpool.tile([128, 128], f32, tag=f"aa_{l}")
                nc.vector.tensor_mul(rn, rz[:, 0:128], P1[:, 256:384])
                n = gpool.tile([128, 128], f32, tag=f"bb_{l}")
                nc.vector.tensor_add(n, rn, P2[:, 256:384])
                nc.scalar.activation(n, n, AF.Tanh)

                # h = n + z*(h_prev - n)
                d = gpool.tile([128, 128], f32, tag=f"aa_{l}")
                if t > 0:
                    nc.vector.tensor_sub(d, own_hL, n)
                else:
                    nc.vector.tensor_scalar_mul(d, n, -1.0)
                zd = gpool.tile([128, 128], f32, tag=f"ca_{l}")
                nc.vector.tensor_mul(zd, z, d)
                hL_new = hpool.tile([128, 128], f32, tag=f"hL_{l}")
                nc.vector.tensor_add(hL_new, n, zd)
                return hL_new

            hT_cur = {1: None, 2: None, 3: None}
            hL_pend = {1: None, 2: None, 3: None}
            for s in range(t_total + 3):
                cons = {}
                for l in (1, 2, 3):
                    t = s - (l - 1)
                    own_hL = hL_pend[l]
                    if own_hL is not None:
                        hT_cur[l] = emit_transpose(l, t - 1, own_hL)
                        hL_pend[l] = None
                    if 0 <= t < t_total:
                        hL_pend[l] = emit_groups(
                            l, t, hT_cur[l - 1] if l > 1 else None,
                            hT_cur[l], own_hL)

        # ---- output phase: s1/s2 heads + masking ------------------------
        with ExitStack() as octx:
            wout = octx.enter_context(tc.tile_pool(name="wout", bufs=1))
            WlT = wout.tile([128, 2, 4, 513], f32r)
            nc.sync.dma_start(out=WlT, in_=WlT_d[:, :, :, :])
            bl = wout.tile([128, 2, 5], f32)
            nc.sync.dma_start(out=bl, in_=bl_d[:, :, :])

            opool = octx.enter_context(tc.tile_pool(name="opool", bufs=3))
            spool = octx.enter_context(tc.tile_pool(name="spool", bufs=2))
            opsum = octx.enter_context(tc.tile_pool(name="opsum", bufs=4, space="PSUM"))

            tc_chunk = min(16, t_total)  # timesteps per chunk -> N = 16*32 = 512
            assert t_total % tc_chunk == 0
            nchunks = t_total // tc_chunk
            if shard_output:
                assert nchunks % NCORES == 0
                nchunks //= NCORES
                pid = nc.partition_id()
            for c in range(nchunks):
                if shard_output:
                    t0 = bass.ds(pid * (nchunks * tc_chunk) + c * tc_chunk,
                                 tc_chunk)
                else:
                    t0 = slice(c * tc_chunk, (c + 1) * tc_chunk)
                rhs = []
                for kt in range(4):
                    rt = opool.tile([128, tc_chunk * 32], f32r, tag=f"rhs{kt}")
                    nc.sync.dma_start(
                        out=rt,
                        in_=h3T[t0, :, 32 * kt:32 * kt + 32]
                        .rearrange("t k b -> k t b"))
                    rhs.append(rt)
                for m in range(5):
                    fp = 128 if m < 4 else 1
                    xt = opool.tile([128, tc_chunk * 32], f32, tag="xchunk")
                    nc.sync.dma_start(
                        out=xt[:fp], in_=xo_d[m, 0:fp, t0, :])
                    ss = []
                    for hd in range(2):
                        ps = opsum.tile([128, tc_chunk * 32], f32, tag=f"ops{hd}")
                        for kt in range(4):
                            nc.tensor.matmul(
                                ps[:fp], WlT[:, hd, kt, m * 128:m * 128 + fp],
                                rhs[kt], start=(kt == 0), stop=(kt == 3))
                        s = spool.tile([128, tc_chunk * 32], f32, tag=f"s{hd}")
                        nc.scalar.activation(
                            s[:fp], ps[:fp], AF.Relu, bias=bl[0:fp, hd, m:m + 1])
                        ss.append(s)
                    # den = (s0 + 1e-16) + s1 in one STT op
                    den = spool.tile([128, tc_chunk * 32], f32, tag="den")
                    nc.vector.scalar_tensor_tensor(
                        den[:fp], ss[0][:fp], 1e-16, ss[1][:fp],
                        ALU.add, ALU.add)
                    rden = spool.tile([128, tc_chunk * 32], f32, tag="rden")
                    nc.vector.reciprocal_approx_fast(rden[:fp], den[:fp])
                    xr = spool.tile([128, tc_chunk * 32], f32, tag="xr")
                    nc.vector.tensor_mul(xr[:fp], xt[:fp], rden[:fp])
                    for hd, out_d in ((0, out1_d), (1, out2_d)):
                        o = spool.tile([128, tc_chunk * 32], f32, tag=f"o{hd}")
                        nc.vector.tensor_mul(o[:fp], ss[hd][:fp], xr[:fp])
                        # issue stores from the scalar HWDGE ring to keep the
                        # sync ring free for the loads
                        nc.scalar.dma_start(
                            out=out_d[m, 0:fp, t0, :],
                            in_=o[:fp].rearrange("f (t b) -> f t b", b=32))

    nc.finalize()
    return nc


# ---------------------------------------------------------------------------
# Entry point
# ---------------------------------------------------------------------------

class _Runner:
    """Caches the compiled PJRT executable so repeat calls only pay
    dispatch + device execution (mirrors bass2jax.run_bass_via_pjrt)."""

    def __init__(self, nc, n_cores):
        import jax
        import concourse.mybir as mybir
        from concourse import bass2jax
        from concourse.bass2jax import (
            _bass_exec_p, install_neuronx_cc_hook, partition_id_tensor)
        from jax.experimental.shard_map import shard_map
        from jax.sharding import Mesh, PartitionSpec

        install_neuronx_cc_hook()
        self.jax = jax
        self.n_cores = n_cores
        partition_name = (nc.partition_id_tensor.name
                          if nc.partition_id_tensor else None)
        in_names, out_names, out_avals, zero_outs = [], [], [], []
        for alloc in nc.m.functions[0].allocations:
            if not isinstance(alloc, mybir.MemoryLocationSet):
                continue
            name = alloc.memorylocations[0].name
            if alloc.kind == "ExternalInput":
                if name != partition_name:
                    in_names.append(name)
            elif alloc.kind == "ExternalOutput":
                shape = tuple(alloc.tensor_shape)
                dtype = mybir.dt.np(alloc.dtype)
                out_names.append(name)
                out_avals.append(jax.core.ShapedArray(shape, dtype))
                zero_outs.append(np.zeros(shape, dtype))
        n_params = len(in_names)
        self.in_names = list(in_names)
        self.out_names = out_names
        self.out_avals = out_avals
        self.zero_outs = zero_outs
        all_in = in_names + out_names
        if partition_name is not None:
            all_in.append(partition_name)

        def _body(*args):
            operands = list(args)
            if partition_name is not None:
                operands.append(partition_id_tensor())
            return tuple(_bass_exec_p.bind(
                *operands, out_avals=tuple(out_avals), in_names=tuple(all_in),
                out_names=tuple(out_names), lowering_input_output_aliases=(),
                sim_require_finite=True, sim_require_nnan=True, nc=nc))

        devices = jax.devices()[:n_cores]
        self.mesh = Mesh(np.asarray(devices), ("core",))
        self.pspec = PartitionSpec("core")
        n_out = len(out_names)
        self.sharded = jax.jit(
            shard_map(_body, mesh=self.mesh,
                      in_specs=(self.pspec,) * (n_params + n_out),
                      out_specs=(self.pspec,) * n_out,
                      check_rep=False),
            keep_unused=True)

    def prepare(self, in_map):
        """Concat per-core inputs + zero out-buffers, device_put once."""
        import jax
        from jax.sharding import NamedSharding
        sh = NamedSharding(self.mesh, self.pspec)
        args = [np.concatenate([np.asarray(in_map[n])] * self.n_cores, axis=0)
                for n in self.in_names]
        args += [np.zeros((self.n_cores * z.shape[0], *z.shape[1:]), z.dtype)
                 for z in self.zero_outs]
        return [jax.device_put(a, sh) for a in args]

    def call(self, concat_in):
        return self.sharded(*concat_in)

    def results_gather(self, outs, t_steps):
        """Assemble full outputs: core c computed time slice c of T."""
        res = {}
        tc = t_steps // self.n_cores
        for i, name in enumerate(self.out_names):
            a = np.asarray(outs[i]).reshape(self.n_cores, *self.out_avals[i].shape)
            res[name] = np.concatenate(
                [a[c][:, :, c * tc:(c + 1) * tc, :] for c in range(self.n_cores)],
                axis=2)
        return res


def _get_runner(t_steps):
    key = (t_steps, True)
    if key not in _CACHE:
        nc = build_nc(t_steps, True)
        _CACHE[key] = _Runner(nc, NCORES)
    return _CACHE[key]


def _run(inputs, t_steps=T, trace=False, time_reps=0):
    import time as _time
    r = _get_runner(t_steps)
    p = prep_inputs(inputs, t_steps)
    concat_in = r.prepare(p)
    outs = r.call(concat_in)  # first call compiles
    out = r.results_gather(outs, t_steps)
    o1 = _unpack_out(out["out1"], t_steps)
    o2 = _unpack_out(out["out2"], t_steps)

    times = []
    for _ in range(time_reps):
        t0 = _time.time()
        outs = r.call(concat_in)
        for o in outs:
            o.block_until_ready()
        times.append(_time.time() - t0)
    return (o1, o2), times


def _unpack_out(o, t_steps):
    """[5, 128, T, B] f-major -> [B, T, 513]."""
    return np.ascontiguousarray(
        np.transpose(o.reshape(640, t_steps, B)[:F], (2, 1, 0)))


def kernel(**inputs):
    (o1, o2), _ = _run(inputs, T)
    return (o1, o2)
